# revision 26
# baseline (speedup 1.0000x reference)
"""LocationSensitiveSoftAttention on 8 Trainium2 NeuronCores (Bass/Tile).

Contract: kernel(**inputs) takes the FULL unsharded inputs (numpy arrays, keys
as in setup_inputs()) and returns the FULL output [64, 1, 256] fp32.

Strategy: data-parallel over batch B=64 -> 8 batches per core; weights
replicated. Math restructure (exact up to fp rounding):
  pre[t,:] = memory[t,:] @ (Wm@We) + conv(state)[t] @ (Wl@We) + r
  r        = (q1 @ Wq) @ We + c0          (c0 folds all biases)
  h = tanh(pre); energy = h @ v_a; s = sigmoid(energy)
  w = state + s/sum(s)
  context = (w @ memory) @ Wm + (sum(state)+1) * bm

Performance structure (per core, 8 batches):
  - The energy path (pre-GEMM, conv part, energy dot) runs in fp8e4m3 with
    DoubleRow perf mode (2 rows/cycle): memory is host-packed into a
    [128e, pair, j, t] interleaved layout; Wm@We is pre-scaled by 16 so its
    entries sit in e4m3's normal range, and tanh un-scales via its input
    scale. The conv is folded into the same PSUM accumulation as 32 extra
    contraction rows (31 shifted copies of padded state, host-packed fp8).
    r enters as tanh's per-partition bias, so no ones-row is needed.
  - The context matvec w @ memory needs full bf16 accuracy (it carries the
    state-dominated output), so memory is also loaded as bf16 in natural
    [t-part, e] layout and contracted on PE (16 chunk matmuls/batch).
  - All big DMAs are host-relaid so every SBUF partition row is one
    contiguous DRAM read (128 descriptors/DMA instead of 2048).
"""

import sys

for _p in ("/root/.axon_site", "/root/.axon_site/_ro/trn_rl_repo",
           "/root/.axon_site/_ro/pypackages", "/opt/trn_rl_repo"):
    if _p not in sys.path:
        sys.path.append(_p)

import numpy as np
import ml_dtypes

B, TQ, T = 64, 2, 2048
HID, ENC, U, FILT, K = 1024, 512, 256, 32, 31
N_CORES = 8
PB = B // N_CORES  # batches per core
PAD = K // 2  # 15
NT = T // 128  # 16 t-tiles

BF16 = ml_dtypes.bfloat16
E4M3 = ml_dtypes.float8_e4m3
SCL = 16.0  # fp8 pre-scaling folded out via activation scale

_BUILT = {}
TRACE = False
LAST_RESULTS = None


def _build_nc(repeat=1):
    import concourse.bacc as bacc
    import concourse.mybir as mybir
    import concourse.tile as tile
    import concourse.bass as bass

    f32 = mybir.dt.float32
    bf16 = mybir.dt.bfloat16
    f8e4 = mybir.dt.float8e4
    AF = mybir.ActivationFunctionType
    ALU = mybir.AluOpType
    AX = mybir.AxisListType
    DR = mybir.MatmulPerfMode.DoubleRow

    nc = bacc.Bacc("TRN2", target_bir_lowering=False, debug=False,
                   num_devices=N_CORES)

    # ---- DRAM I/O ----
    # nat[p, ti, e] = mem[b, ti*128+p, e]; t-chunks 0..10 bf16, 11..15 fp8
    NBF = 11
    natb_d = nc.dram_tensor("natb", [PB, 128, NBF, ENC], bf16, kind="ExternalInput")
    nat8_d = nc.dram_tensor("nat8", [PB, 128, NT - NBF, ENC], f8e4,
                            kind="ExternalInput")
    # m8[ew, pair, j, t] = fp8(mem[b, t, (2*pair+j)*128+ew])
    m8_d = nc.dram_tensor("m8", [PB, 128, 2, 2, T], f8e4, kind="ExternalInput")
    # sh8[k, j, t] = fp8(spad[b, t + 16*j + k])   (31 shifts + zero slot)
    sh8_d = nc.dram_tensor("sh8", [PB, 16, 2, T], f8e4, kind="ExternalInput")
    # stTall[p, b, ci] = bf16(state[b, ci*128+p])
    stT_d = nc.dram_tensor("stT", [128, PB, NT], bf16, kind="ExternalInput")
    statef_d = nc.dram_tensor("statef", [PB, T], f32, kind="ExternalInput")
    # wmwe8[ew, pair, j, u] = fp8(16 * (Wm@We)[(2*pair+j)*128+ew, u])
    wmwe8_d = nc.dram_tensor("wmwe8", [128, 2, 2, U], f8e4, kind="ExternalInput")
    # g16[k, j, u] = fp8(16 * G[16*j+k, u]), G[s,u] = sum_f conv_w[f,0,s]*WlWe[f,u]
    g16_d = nc.dram_tensor("g16", [16, 2, U], f8e4, kind="ExternalInput")
    # q1T8[ew, pr, j, b] = fp8(q1[b, (2*pr+j)*128+ew])
    q1T8_d = nc.dram_tensor("q1T8", [128, 4, 2, 32], f8e4, kind="ExternalInput")
    # wq8[ew, pr, j, u] = fp8(16 * Wq[(2*pr+j)*128+ew, u])
    wq8_d = nc.dram_tensor("wq8", [128, 4, 2, U], f8e4, kind="ExternalInput")
    we_d = nc.dram_tensor("we", [U, U], bf16, kind="ExternalInput")
    c0_d = nc.dram_tensor("c0", [1, U], bf16, kind="ExternalInput")
    # va16[ew, j, 0] = fp8(16 * v_a[j*128+ew])
    va16_d = nc.dram_tensor("va16", [128, 2, 1], f8e4, kind="ExternalInput")
    wm_d = nc.dram_tensor("wm", [ENC, U], bf16, kind="ExternalInput")
    bm_d = nc.dram_tensor("bm", [1, U], f32, kind="ExternalInput")
    idb_d = nc.dram_tensor("idb", [128, 128], bf16, kind="ExternalInput")
    idf_d = nc.dram_tensor("idf", [PB, PB], f32, kind="ExternalInput")
    out_d = nc.dram_tensor("out", [PB, U], f32, kind="ExternalOutput")

    with tile.TileContext(nc) as tc:
        with (
            tc.tile_pool(name="consts", bufs=1) as consts,
            tc.tile_pool(name="nat", bufs=4) as natp,
            tc.tile_pool(name="m8p", bufs=3) as m8p,
            tc.tile_pool(name="hp", bufs=2) as hp,
            tc.tile_pool(name="rows", bufs=3) as rowp,
            tc.tile_pool(name="psA", bufs=3, space="PSUM") as psA,
            tc.tile_pool(name="psB", bufs=1, space="PSUM") as psB,
            tc.tile_pool(name="psC", bufs=1, space="PSUM") as psC,
        ):
          def _body():
              # ---- load constants ----
              q1T8_sb = consts.tile([128, 4, 2, 32], f8e4, tag="q1T8")
              nc.scalar.dma_start(out=q1T8_sb[:], in_=q1T8_d.ap())
              wq8_sb = consts.tile([128, 4, 2, U], f8e4, tag="wq8")
              nc.scalar.dma_start(out=wq8_sb[:], in_=wq8_d.ap())
              we_sb = []
              for j in range(2):
                  t_ = consts.tile([128, U], bf16, tag=f"we{j}")
                  nc.scalar.dma_start(out=t_[:], in_=we_d.ap()[j * 128:(j + 1) * 128, :])
                  we_sb.append(t_)
              c0_sb = consts.tile([1, U], bf16, tag="c0")
              nc.scalar.dma_start(out=c0_sb[:], in_=c0_d.ap())
              wmwe8_sb = consts.tile([128, 2, 2, U], f8e4, tag="wmwe8")
              nc.scalar.dma_start(out=wmwe8_sb[:], in_=wmwe8_d.ap())
              g16_sb = consts.tile([16, 2, U], f8e4, tag="g16")
              nc.scalar.dma_start(out=g16_sb[:], in_=g16_d.ap())
              va16_sb = consts.tile([128, 2, 1], f8e4, tag="va16")
              nc.scalar.dma_start(out=va16_sb[:], in_=va16_d.ap())
              idb_sb = consts.tile([128, 128], bf16, tag="idb")
              nc.scalar.dma_start(out=idb_sb[:], in_=idb_d.ap())
              stT_sb = consts.tile([128, PB, NT], bf16, tag="stT")
              nc.sync.dma_start(out=stT_sb[:], in_=stT_d.ap())
              wm_sb = []
              for ec in range(4):
                  t_ = consts.tile([128, U], bf16, tag=f"wm{ec}")
                  nc.sync.dma_start(out=t_[:], in_=wm_d.ap()[ec * 128:(ec + 1) * 128, :])
                  wm_sb.append(t_)
              bm_sb = consts.tile([1, U], f32, tag="bm")
              nc.sync.dma_start(out=bm_sb[:], in_=bm_d.ap())
              idf_sb = consts.tile([PB, PB], f32, tag="idf")
              nc.sync.dma_start(out=idf_sb[:], in_=idf_d.ap())
              statef_sb = consts.tile([PB, T], f32, tag="statef")
              nc.sync.dma_start(out=statef_sb[:], in_=statef_d.ap())
              ones8 = consts.tile([1, PB], bf16, tag="ones8")
              nc.vector.memset(ones8[:], 1.0)
              ones128 = consts.tile([128, 1], f32, tag="ones128")
              nc.vector.memset(ones128[:], 1.0)
              call2 = consts.tile([2 * PB, ENC], bf16, tag="call2")
              ssum_row = consts.tile([1, PB], f32, tag="ssumrow")

              # ---- r rows: pq = q1@Wq (fp8 DR), rT = We^T pq^T + c0
              # all transposes on PE (is_transpose) to avoid DMA latency
              idf8 = consts.tile([PB, PB], bf16, tag="idf8")
              nc.vector.memset(idf8[:], 0.0)
              pq_ps = psC.tile([32, U], f32, tag="cv", name="pq_ps")
              for pr in range(4):
                  nc.tensor.matmul(pq_ps[:], q1T8_sb[:, pr, :, :],
                                   wq8_sb[:, pr, :, :],
                                   start=(pr == 0), stop=(pr == 3), perf_mode=DR)
              pq_bf = consts.tile([32, U], bf16, tag="pqbf")
              nc.vector.tensor_scalar(out=pq_bf[:], in0=pq_ps[:],
                                      scalar1=1.0 / SCL, scalar2=None, op0=ALU.mult)
              pqT2 = consts.tile([128, 2, 32], bf16, tag="pqT2")
              for kc in range(2):
                  pqT_ps = psB.tile([128, 32], bf16, tag="enps",
                                    name=f"pqT_ps{kc}")
                  nc.tensor.matmul(pqT_ps[:], pq_bf[:, kc * 128:(kc + 1) * 128],
                                   idb_sb[0:32, 0:32], is_transpose=True)
                  nc.vector.tensor_copy(pqT2[:, kc, :], pqT_ps[:])
              rT = consts.tile([128, 2, PB], bf16, tag="rT")
              for vch in range(2):
                  rT_ps = psB.tile([128, PB], f32, tag="enps",
                                   name=f"rT_ps{vch}")
                  for kc in range(2):
                      nc.tensor.matmul(rT_ps[:],
                                       we_sb[kc][:, vch * 128:(vch + 1) * 128],
                                       pqT2[:, kc, 0:PB],
                                       start=(kc == 0), stop=False)
                  nc.tensor.matmul(rT_ps[:],
                                   c0_sb[:, vch * 128:(vch + 1) * 128],
                                   ones8[:], start=False, stop=True)
                  nc.vector.tensor_copy(rT[:, vch, :], rT_ps[:])

              # ---- per-batch main loop ----
              def load_b(b):
                  st = {}
                  m8 = m8p.tile([128, 2, 2, T], f8e4, tag="m8", name=f"m8_{b}")
                  nc.gpsimd.dma_start(out=m8[:], in_=m8_d.ap()[b])
                  sh8 = m8p.tile([16, 2, T], f8e4, tag="sh8", name=f"sh8_{b}")
                  nc.gpsimd.dma_start(out=sh8[:], in_=sh8_d.ap()[b])
                  natb = natp.tile([128, NBF, ENC], bf16, tag="nat",
                                   name=f"natb{b}")
                  nc.gpsimd.dma_start(out=natb[:], in_=natb_d.ap()[b])
                  nat8 = natp.tile([128, NT - NBF, ENC], f8e4, tag="nat8",
                                   name=f"nat8_{b}")
                  nc.gpsimd.dma_start(out=nat8[:], in_=nat8_d.ap()[b])
                  st.update(natb=natb, nat8=nat8, m8=m8, sh8=sh8)
                  return st

              def stage1a(b, st):
                  m8, sh8 = st["m8"], st["sh8"]
                  h8 = hp.tile([128, 2, 4, 512], f8e4, tag="h8",
                               name=f"h8_{b}")
                  st["h8"] = h8
                  for half in range(2):
                      for vch in range(2):
                          pre_t = psA.tile([128, 2, 512], f32, tag="pre",
                                           name=f"pre_{vch}{half}_{b}")
                          for pair in range(2):
                              lw = wmwe8_sb[:, pair, :, vch * 128:(vch + 1) * 128]
                              for ti in range(2):
                                  tb = 2 * half + ti
                                  nc.tensor.matmul(
                                      pre_t[:, ti, :], lw,
                                      m8[:, pair, :, tb * 512:(tb + 1) * 512],
                                      start=(pair == 0), stop=False,
                                      perf_mode=DR, skip_group_check=True)
                          lg = g16_sb[:, :, vch * 128:(vch + 1) * 128]
                          for ti in range(2):
                              tb = 2 * half + ti
                              nc.tensor.matmul(
                                  pre_t[:, ti, :], lg,
                                  sh8[:, :, tb * 512:(tb + 1) * 512],
                                  start=False, stop=True,
                                  perf_mode=DR, skip_group_check=True)
                          nc.scalar.activation(
                              h8[:, vch, 2 * half:2 * half + 2, :], pre_t[:],
                              AF.Tanh, bias=rT[:, vch, b:b + 1],
                              scale=1.0 / SCL)

              def stage1b(b, st):
                  h8, natb, nat8 = st["h8"], st["natb"], st["nat8"]
                  enT_ps = psB.tile([128, NT], f32, tag="enps",
                                    name=f"enT{b}")
                  wst = rowp.tile([128, NT, 2], bf16, tag="wst",
                                  name=f"wst{b}")
                  st["wst"] = wst
                  wst_t = bass.AP(tensor=wst.tensor, offset=wst.offset + 1,
                                  ap=[wst.ap[0], [2, NT]])
                  nc.vector.tensor_copy(wst_t, stT_sb[:, b, :])
                  cv_ps = psC.tile([2, ENC], f32, tag="cv", name=f"cv{b}")
                  st["cv_ps"] = cv_ps
                  saccs = []
                  def en_chunk(ci):
                      tb, tw = ci // 4, ci % 4
                      nc.tensor.matmul(
                          enT_ps[:, ci:ci + 1],
                          h8[:, :, tb, tw * 128:(tw + 1) * 128],
                          va16_sb[:], start=True, stop=True, perf_mode=DR)

                  def cv_chunk(ci):
                      mv = (natb[:, ci, :] if ci < NBF
                            else nat8[:, ci - NBF, :])
                      nc.tensor.matmul(cv_ps[:], wst[:, ci, :], mv,
                                       start=(ci == 0), stop=(ci == NT - 1),
                                       skip_group_check=True)

                  def sig_half(half):
                      wst_sh = bass.AP(
                          tensor=wst.tensor, offset=wst.offset + 16 * half,
                          ap=[wst.ap[0], [2, 8]])
                      sacc = rowp.tile([128, 1], f32, tag="sacc",
                                       name=f"sacc{half}_{b}", bufs=4)
                      saccs.append(sacc)
                      nc.scalar.activation(wst_sh, enT_ps[:, 8 * half:8 * half + 8],
                                           AF.Sigmoid, scale=1.0 / SCL,
                                           accum_out=sacc[:])

                  for ci in range(8):
                      en_chunk(ci)
                  sig_half(0)
                  for i in range(8):
                      cv_chunk(i)
                  for i in range(8):
                      en_chunk(8 + i)
                  sig_half(1)
                  for i in range(8, 16):
                      cv_chunk(i)
                  ssb_ps = psB.tile([1, 1], f32, tag="enps", name=f"ssb{b}")
                  nc.tensor.matmul(ssb_ps[:], saccs[0][:], ones128[:],
                                   start=True, stop=False)
                  nc.tensor.matmul(ssb_ps[:], saccs[1][:], ones128[:],
                                   start=False, stop=True)
                  nc.vector.tensor_copy(ssum_row[:, b:b + 1], ssb_ps[:])

              def stage2(b, st):
                  cv_ps = st["cv_ps"]
                  cv_sb = rowp.tile([2, ENC], bf16, tag="cvsb", name=f"cvsb{b}")
                  nc.vector.tensor_copy(cv_sb[:], cv_ps[:])
                  # rows b (s-part) and PB+b (state-part) of call2
                  nc.sync.dma_start(out=call2[b:b + 1, :], in_=cv_sb[0:1, :])
                  nc.sync.dma_start(out=call2[PB + b:PB + b + 1, :],
                                    in_=cv_sb[1:2, :])

              st = load_b(0)
              prev = None
              for b in range(PB):
                  st_next = load_b(b + 1) if b + 1 < PB else None
                  stage1a(b, st)
                  if prev is not None:
                      stage2(*prev)
                  stage1b(b, st)
                  prev = (b, st)
                  st = st_next
              stage2(*prev)

              # ---- sum(state) + 1 row [1, PB] ----
              stsum = consts.tile([PB, 1], f32, tag="stsum")
              nc.vector.tensor_reduce(stsum[:], statef_sb[:], axis=AX.X, op=ALU.add)
              ps_sig = psC.tile([1, PB], f32, tag="cv", name="ps_sig")
              nc.tensor.matmul(ps_sig[:], stsum[:], idf_sb[:], is_transpose=True)
              sig_row = consts.tile([1, PB], f32, tag="sigrow")
              nc.vector.tensor_scalar_add(sig_row[:], ps_sig[:], 1.0)

              # ---- final: ctx = recs*(callS@Wm) + callState@Wm + sig^T*bm
              ssc = consts.tile([PB, 1], f32, tag="ssc")
              nc.sync.dma_start(out=ssc[:], in_=ssum_row[:])
              recs8 = consts.tile([PB, 1], f32, tag="recs8")
              nc.vector.reciprocal(recs8[:], ssc[:])
              callT = []
              for ch in range(4):
                  pst = psB.tile([128, 2 * PB], bf16, tag="enps",
                                 name=f"callTps{ch}")
                  nc.tensor.matmul(pst[:], call2[:, ch * 128:(ch + 1) * 128],
                                   idb_sb[0:2 * PB, 0:2 * PB],
                                   is_transpose=True)
                  t_ = consts.tile([128, 2 * PB], bf16, tag=f"callT{ch}")
                  nc.vector.tensor_copy(t_[:], pst[:])
                  callT.append(t_)
              ctxS_ps = psC.tile([PB, U], f32, tag="cv", name="ctxS")
              ctxT_ps = psB.tile([PB, U], f32, tag="enps", name="ctxT")
              for ch in range(4):
                  nc.tensor.matmul(ctxS_ps[:], callT[ch][:, 0:PB], wm_sb[ch][:],
                                   start=(ch == 0), stop=(ch == 3))
                  nc.tensor.matmul(ctxT_ps[:], callT[ch][:, PB:2 * PB],
                                   wm_sb[ch][:],
                                   start=(ch == 0), stop=False)
              nc.tensor.matmul(ctxT_ps[:], sig_row[:], bm_sb[:],
                               start=False, stop=True)
              ctxT_sb = consts.tile([PB, U], f32, tag="ctxT")
              nc.vector.tensor_copy(ctxT_sb[:], ctxT_ps[:])
              ctx_sb = consts.tile([PB, U], f32, tag="ctx")
              nc.vector.scalar_tensor_tensor(
                  ctx_sb[:], in0=ctxS_ps[:], scalar=recs8[:], in1=ctxT_sb[:],
                  op0=ALU.mult, op1=ALU.add)
              nc.sync.dma_start(out=out_d.ap(), in_=ctx_sb[:])

          for _rep in range(repeat):
              _body()
    nc.compile()
    return nc


def _host_prep(inputs):
    """Weight folds (weight-only transforms) + per-core layout/dtype shards."""
    f32 = np.float32
    Wq = np.asarray(inputs["Wq"], f32)
    bq = np.asarray(inputs["bq"], f32)
    Wm = np.asarray(inputs["Wm"], f32)
    bm = np.asarray(inputs["bm"], f32)
    Wl = np.asarray(inputs["Wl"], f32)
    bl = np.asarray(inputs["bl"], f32)
    conv_w = np.asarray(inputs["conv_w"], f32)
    conv_b = np.asarray(inputs["conv_b"], f32)
    We = np.asarray(inputs["We"], f32)
    be = np.asarray(inputs["be"], f32)
    v_a = np.asarray(inputs["v_a"], f32)

    WmWe = Wm @ We
    WlWe = Wl @ We
    c0 = ((bq + bm + bl) @ We + be + conv_b @ WlWe).astype(f32)
    # G[s, u] = sum_f conv_w[f, 0, s] * WlWe[f, u]; slot 31 zeroed
    G = np.zeros((32, U), f32)
    G[:K] = conv_w[:, 0, :].T @ WlWe

    query = np.asarray(inputs["query"], f32)
    state = np.asarray(inputs["state"], f32)
    memory = np.asarray(inputs["memory"], f32)

    spad = np.zeros((B, T + 2 * PAD), f32)
    spad[:, PAD:PAD + T] = state
    q1 = query[:, 1, :]

    ident = np.eye(128, dtype=f32)
    shared = {
        "wmwe8": np.ascontiguousarray(
            (SCL * WmWe).reshape(2, 2, 128, U).transpose(2, 0, 1, 3)).astype(E4M3),
        "g16": np.ascontiguousarray(
            (SCL * G).reshape(2, 16, U).transpose(1, 0, 2)).astype(E4M3),
        "wq8": np.ascontiguousarray(
            (SCL * Wq).reshape(4, 2, 128, U).transpose(2, 0, 1, 3)).astype(E4M3),
        "we": We.astype(BF16),
        "c0": c0.reshape(1, U).astype(BF16),
        "va16": np.ascontiguousarray(
            (SCL * v_a).reshape(2, 128, 1).transpose(1, 0, 2)).astype(E4M3),
        "wm": Wm.astype(BF16),
        "bm": bm.reshape(1, U).astype(f32),
        "idb": ident.astype(BF16),
        "idf": ident[:PB, :PB].copy(),
    }
    # sh8 shift index s = 16*j + k, s in [0, 31); slot 31 multiplies G row 31=0
    shift_idx = np.arange(32).reshape(2, 16)  # [j, k]
    in_maps = []
    for c in range(N_CORES):
        sl = slice(c * PB, (c + 1) * PB)
        m = dict(shared)
        mb = memory[sl]  # [PB, T, ENC] f32
        natf = np.ascontiguousarray(
            mb.reshape(PB, NT, 128, ENC).transpose(0, 2, 1, 3))
        m["natb"] = natf[:, :, :11, :].astype(BF16)
        m["nat8"] = np.ascontiguousarray(natf[:, :, 11:, :]).astype(E4M3)
        m["m8"] = np.ascontiguousarray(
            mb.reshape(PB, T, 2, 2, 128).transpose(0, 4, 2, 3, 1)).astype(E4M3)
        sp = spad[sl]  # [PB, T+30]
        sh = np.empty((PB, 16, 2, T), f32)
        for j in range(2):
            for k in range(16):
                s = shift_idx[j, k]
                if s < 31:
                    sh[:, k, j, :] = sp[:, s:s + T]
                else:
                    sh[:, k, j, :] = 0.0
        m["sh8"] = sh.astype(E4M3)
        stc = state[sl]  # [PB, T]
        m["stT"] = np.ascontiguousarray(
            stc.reshape(PB, NT, 128).transpose(2, 0, 1)).astype(BF16)
        m["statef"] = np.ascontiguousarray(stc)
        q1p = np.zeros((32, HID), np.float32)
        q1p[:PB] = q1[sl]
        m["q1T8"] = np.ascontiguousarray(
            q1p.reshape(32, 4, 2, 128).transpose(3, 1, 2, 0)).astype(E4M3)
        in_maps.append(m)
    return in_maps


def kernel(**inputs) -> np.ndarray:
    global LAST_RESULTS
    from concourse import bass_utils

    if "nc" not in _BUILT:
        _BUILT["nc"] = _build_nc()
    nc = _BUILT["nc"]

    in_maps = _host_prep(inputs)
    res = bass_utils.run_bass_kernel_spmd(
        nc, in_maps, core_ids=list(range(N_CORES)), trace=TRACE)
    LAST_RESULTS = res
    out = np.concatenate([res.results[c]["out"] for c in range(N_CORES)], axis=0)
    return out.reshape(B, 1, U).astype(np.float32)
